# revision 10
# baseline (speedup 1.0000x reference)
"""Trainium2 Bass kernel for a 2-layer LSTM encoder + autoregressive decoder.

Problem: batch 8192, 48 encode steps, 12 decode steps with BG-channel
feedback, hidden 128, input dim 8, fc head to 1 output.

Strategy: pure data parallelism over 8 NeuronCores (1024 batch rows each).
Per core the recurrence runs sequentially; within a step, hidden units sit
on SBUF partitions and batch (1024) on the free dim:

  gates[512, B] = W_ih^T.T @ x[8, B] + W_hh^T.T @ h[128, B]   (PSUM accum)

Matmuls run in fp16 (fp32 PSUM accumulation). Per-gate biases ride the
activation instruction's per-partition bias operand (fp32, exact), so x
carries no ones-channel.

Engine balance per layer-step (the scalar engine is the transcendental
bottleneck, so two of its ops are offloaded):
  ScalarE: tanh(g), sig(i), sig(f), sig(o)          [4 activations]
  DVE:     i*g, f*c, c=+, tanh(c) as 2 custom ops   [rational approx]
  Pool:    h = o * tanh(c)                          [GPSIMD tensor_mul]

tanh(c) uses a minimax rational clamp(s*y*(y^2+n0)/(y^2+d0)) with the
reciprocal from a BITWISE_NOT exponent-flip seed + 1 Newton step, fit to
max abs err 4.5e-3 -- two 8-stage custom DVE ops registered at runtime.

Decode BG feedback: pred_{t-1} = W_fc h1_{t-1} + b_fc is folded through
the BG column as the rank-1 weight wbgfc = outer(W_fc, W_bg) applied to
last-step h1; b_fc * W_bg is folded into the decode-phase bias tile.
b_fc itself is added host-side after the gather.
"""

import sys

sys.path.insert(0, "/opt/trn_rl_repo")

import numpy as np
import ml_dtypes

import concourse.bacc as bacc
import concourse.tile as tile
from concourse import mybir
from concourse import bass_utils
from concourse.bass import ts

F16 = np.float16

B_TOTAL = 8192
T = 60
T_ENC = 48
T_DEC = 12
DIN = 8
H = 128
NG = 4 * H
N_CORES = 8
BSH = B_TOTAL // N_CORES  # 1024 batch rows per core
NS = 2  # batch halves (matmul moving-dim limit is 512)
SB = BSH // NS  # 512
XT_STEPS = 8  # timesteps per streamed x tile

# pytorch gate order in the weight columns: i, f, g, o
GI, GF, GG, GO = 0, 1, 2, 3

# minimax fit of clamp(s*y*(y^2+n0)*recip1nr(y^2+d0)) to tanh(y):
# max abs err 4.5e-3 over [-6, 6] (saturates exactly beyond)
TP_N0 = 18.67619716
TP_D0 = 2.59404445
TP_S = 0.13342206
TP_RC0 = -0.24172069  # recip seed scale (on BITWISE_NOT exponent flip)
TP_RC1 = 2.04844232  # tuned Newton constant

_CACHE: dict = {}


def _register_tanh_ops():
    """Register the two custom DVE ops implementing the rational tanh."""
    if "ops" in _CACHE:
        return _CACHE["ops"]
    from concourse import dve_ops
    from concourse.dve_spec import (
        Spec, Src0, Src1, C0, C1, C2, Zero, One, AluOp, Bin,
        maxx, minn, sq, lower, _has_src1,
    )
    from concourse.dve_uop import DveOpSpec

    def register(name, spec, subdim=False):
        row = dve_ops._CUSTOM_DVE_ROW_BASE + len(dve_ops.OPS)
        dve_ops._SUB_OPCODE_FOR_NAME[name] = row
        shas = {}
        for ver in ("v3",):
            uops = lower(spec, ver=ver)
            shas[ver] = DveOpSpec(
                name=name, opcode=row, uops=uops, rd1_en=_has_src1(spec)
            ).sha(ver)
        op = dve_ops.DveOp(name, spec, subdim=subdim, uops_sha=shas)
        dve_ops.OPS.append(op)
        dve_ops.CUSTOM_DVE_SPECS[name] = spec
        return op

    def _np_recip(in0, in1, s0, s1, imm2):
        d = (in0.astype(np.float32) ** 2 + s0).astype(np.float32)
        nd = (~d.view(np.int32)).view(np.float32)
        y0 = nd * s1
        return y0 * (imm2 - d * y0)

    # op1: r = recip-1NR(y^2 + d0); s0=d0, s1=seed scale, imm2=newton const
    q1 = sq(Src0)
    d = q1 + C0
    nd = Bin(AluOp.BITWISE_NOT, d, d)
    y0 = nd * C1
    r_body = y0 * (C2 - d * y0)
    op_recip = register(
        "LSTM_TANH_RECIP_ANT",
        Spec(body=r_body, reference=_np_recip),
    )

    def _np_finish(in0, in1, s0, s1, imm2):
        t = (in0.astype(np.float32) ** 2 + s0) * in0 * in1 * s1
        return np.clip(t, -1.0, 1.0).astype(np.float32)

    # op2: t = clamp(s*y*(y^2+n0)*r, -1, 1); in1=r, s0=n0, s1=s
    q2 = sq(Src0)
    t_body = maxx(minn(((q2 + C0) * Src0) * Src1 * C1, One), Zero - One)
    op_fin = register(
        "LSTM_TANH_FINISH_ANT",
        Spec(body=t_body, reference=_np_finish),
    )
    _CACHE["ops"] = (op_recip, op_fin)
    return _CACHE["ops"]


def _build():
    f32 = mybir.dt.float32
    f16 = mybir.dt.float16

    op_recip, op_fin = _register_tanh_ops()

    nc = bacc.Bacc("TRN2", debug=False, num_devices=N_CORES)

    x_d = nc.dram_tensor("x", [DIN, T, BSH], f16, kind="ExternalInput")
    w0x_d = nc.dram_tensor("w0x", [DIN, NG], f16, kind="ExternalInput")
    w0h_d = nc.dram_tensor("w0h", [H, NG], f16, kind="ExternalInput")
    w1x_d = nc.dram_tensor("w1x", [H, NG], f16, kind="ExternalInput")
    w1h_d = nc.dram_tensor("w1h", [H, NG], f16, kind="ExternalInput")
    wbgfc_d = nc.dram_tensor("wbgfc", [H, NG], f16, kind="ExternalInput")
    wfc_d = nc.dram_tensor("wfc", [H, 1], f16, kind="ExternalInput")
    b0e_d = nc.dram_tensor("b0e", [H, 4], f32, kind="ExternalInput")
    b0d_d = nc.dram_tensor("b0d", [H, 4], f32, kind="ExternalInput")
    b1_d = nc.dram_tensor("b1", [H, 4], f32, kind="ExternalInput")
    out_d = nc.dram_tensor("out", [T_DEC, BSH], f32, kind="ExternalOutput")

    SIG = mybir.ActivationFunctionType.Sigmoid
    TANH = mybir.ActivationFunctionType.Tanh

    with tile.TileContext(nc) as tc:
        with (
            tc.tile_pool(name="wpool", bufs=1) as wpool,
            tc.tile_pool(name="xpool", bufs=3) as xpool,
            tc.tile_pool(name="state", bufs=1) as state,
            tc.tile_pool(name="gates", bufs=3) as gates,
            tc.tile_pool(name="psum", bufs=2, space="PSUM") as psum,
        ):
            w0x = wpool.tile([DIN, NG], f16)
            w0h = wpool.tile([H, NG], f16)
            w1x = wpool.tile([H, NG], f16)
            w1h = wpool.tile([H, NG], f16)
            wbgfc = wpool.tile([H, NG], f16)
            wfc = wpool.tile([H, 1], f16)
            b0e = wpool.tile([H, 4], f32)
            b0d = wpool.tile([H, 4], f32)
            b1 = wpool.tile([H, 4], f32)
            for tl, d in (
                (w0x, w0x_d), (w0h, w0h_d), (w1x, w1x_d), (w1h, w1h_d),
                (wbgfc, wbgfc_d), (wfc, wfc_d),
                (b0e, b0e_d), (b0d, b0d_d), (b1, b1_d),
            ):
                nc.sync.dma_start(tl[:], d.ap())

            # recurrent state; h double-buffered on step parity
            h = [[None, None], [None, None]]  # h[layer][parity]
            c = [None, None]  # c[layer]
            for l in range(2):
                for p in range(2):
                    h[l][p] = state.tile([H, NS, SB], f16, name=f"h_{l}_{p}")
                    nc.vector.memset(h[l][p][:], 0.0)
                c[l] = state.tile([H, NS, SB], f16, name=f"c_{l}")
                nc.vector.memset(c[l][:], 0.0)

            def layer_mm(t, layer, xt, tr):
                """Emit this layer's matmuls, gate-major.

                Each gate gets its own 2-bank PSUM tile [128, stream, 512];
                four tiles are in flight per layer (2 tags x 2 bufs = all 8
                banks), so PSUM frees one sigma/tanh op at a time and the
                tensor engine never faces a whole-layer drain bubble.
                Within a gate, the part whose rhs only needs last-step state
                goes first so the scheduler can prefetch it.
                """
                p = t % 2
                dec = t > T_ENC

                def x_sl(st):
                    if layer == 0:
                        return xt[:, tr, ts(st, SB)]
                    return h[0][p][:, st, :]

                def h_sl(st):
                    return h[layer][1 - p][:, st, :]

                if layer == 0:
                    # x-part first: x is static data, so these matmuls run
                    # while h0(t-1) is still being produced; the h-part goes
                    # last so act(g) fires one matmul after h0 lands
                    parts = [(w0x, x_sl), (w0h, h_sl)]
                    if dec:
                        # decode: pred_{t-1} = W_fc h1_{t-1} + b_fc is folded
                        # through the BG column as the rank-1 weight
                        # wbgfc = outer(W_fc, W_bg) applied to last-step h1
                        parts.insert(
                            1, (wbgfc, lambda st: h[1][1 - p][:, st, :])
                        )
                else:
                    # h-part first: it only needs last-step h1 (prefetchable);
                    # the x-part (= this step's h0) is the late dependency
                    parts = [(w1h, h_sl), (w1x, x_sl)]

                # g first (it unblocks the DVE chain), then i, f, o
                gps = {}
                for g, tag in [(GG, "psA"), (GI, "psB"), (GF, "psA"), (GO, "psB")]:
                    gps[g] = psum.tile(
                        [H, NS, SB], f32, tag=tag, name=f"ps_{t}_{layer}_{g}"
                    )
                for pi, (w, rhs_fn) in enumerate(parts):
                    for g in (GG, GI, GF, GO):
                        for st in range(NS):
                            nc.tensor.matmul(
                                gps[g][:, st, :], w[:, ts(g, H)], rhs_fn(st),
                                start=pi == 0, stop=pi == len(parts) - 1,
                            )
                return gps

            def layer_act_dve(t, layer, gps):
                p = t % 2
                h_new = h[layer][p]
                c_own = c[layer]
                bias = b1 if layer == 1 else (b0d if t > T_ENC else b0e)

                ifo_sb = gates.tile([H, 3, NS, SB], f16, tag="ifo")
                g_sb = gates.tile([H, NS, SB], f16, tag="g")
                t1 = gates.tile([H, NS, SB], f16, tag="t1")
                u = gates.tile([H, NS, SB], f16, tag="u")
                th = gates.tile([H, NS, SB], f16, tag="th")
                rcp = gates.tile([H, NS, SB], f16, tag="rcp")

                def act(dst, g, func):
                    nc.scalar.activation(
                        dst, gps[g][:], func, bias=bias[:, g : g + 1]
                    )

                act(g_sb[:], GG, TANH)
                act(ifo_sb[:, 0], GI, SIG)
                act(ifo_sb[:, 1], GF, SIG)
                nc.vector.tensor_mul(t1[:], ifo_sb[:, 0], g_sb[:])
                nc.vector.tensor_mul(u[:], ifo_sb[:, 1], c_own[:])
                nc.vector.tensor_add(c_own[:], u[:], t1[:])
                act(ifo_sb[:, 2], GO, SIG)
                # tanh(c): stream 0 on ScalarE, stream 1 as two custom DVE
                # ops (rational + seeded recip)
                nc.scalar.activation(th[:, 0], c_own[:, 0], TANH)
                nc.vector._custom_dve(
                    op_recip, out=rcp[:, 1], in0=c_own[:, 1],
                    s0=TP_D0, s1=TP_RC0, imm2=TP_RC1,
                )
                nc.vector._custom_dve(
                    op_fin, out=th[:, 1], in0=c_own[:, 1], in1=rcp[:, 1],
                    s0=TP_N0, s1=TP_S,
                )
                # h = o * tanh(c), per stream; stream 1 on the idle Pool
                # engine so the DVE only carries the stream-0 path
                nc.vector.tensor_mul(h_new[:, 0], ifo_sb[:, 2, 0], th[:, 0])
                nc.gpsimd.tensor_mul(h_new[:, 1], ifo_sb[:, 2, 1], th[:, 1])

            def fc_block(t):
                td = t - T_ENC
                fc = psum.tile([1, NS, SB], f32, tag="psA", name=f"fc_{t}")
                for st in range(NS):
                    nc.tensor.matmul(
                        fc[:, st, :], wfc[:], h[1][t % 2][:, st, :],
                        start=True, stop=True,
                    )
                # raw fc output to HBM; b_fc is added host-side
                pred = gates.tile([1, NS, SB], f32, tag="pred")
                nc.vector.tensor_scalar_add(pred[:], fc[:], 0.0)
                nc.sync.dma_start(out_d.ap()[td : td + 1, :], pred[:])

            # x tiles: 8-step tiles for encode, one 12-step tile for decode
            xt = None
            t0 = 0
            x_tile_starts = list(range(0, T_ENC, XT_STEPS)) + [T_ENC]
            for t in range(T):
                if t in x_tile_starts:
                    t0 = t
                    nt = T_DEC if t == T_ENC else XT_STEPS
                    xt = xpool.tile([DIN, T_DEC, BSH], f16)
                    nc.sync.dma_start(
                        xt[:, :nt, :], x_d.ap()[:, t : t + nt, :]
                    )
                tr = t - t0
                for layer in range(2):
                    gps = layer_mm(t, layer, xt, tr)
                    layer_act_dve(t, layer, gps)
                if t >= T_ENC:
                    fc_block(t)

    nc.compile()
    return nc


def _get_nc():
    if "nc" not in _CACHE:
        _CACHE["nc"] = _build()
    return _CACHE["nc"]


def kernel(
    inputs,
    W_ih_0, W_hh_0, b_ih_0, b_hh_0,
    W_ih_1, W_hh_1, b_ih_1, b_hh_1,
    W_fc, b_fc,
):
    inputs = np.asarray(inputs, np.float32)
    nc = _get_nc()

    bfc = np.float32(np.asarray(b_fc).reshape(-1)[0])
    b0 = (b_ih_0 + b_hh_0).astype(np.float32)
    wbg = W_ih_0[:, 0].astype(np.float32)  # BG column of W_ih_0

    w0x = W_ih_0.T.astype(F16)  # [8, 512]
    w0h = W_hh_0.T.astype(F16)
    w1x = W_ih_1.T.astype(F16)
    w1h = W_hh_1.T.astype(F16)
    # rank-1 fold of the fc head through the BG column: gate j gets
    # W_ih_0[j,0] * (W_fc . h1); lhsT[k, j] = W_fc[0,k] * W_ih_0[j,0]
    wbgfc = np.outer(W_fc.astype(np.float32)[0], wbg).astype(F16)
    wfc = W_fc.T.astype(F16)  # [128, 1]

    b0e = b0.reshape(4, H).T.astype(np.float32)  # [128, 4]
    # decode bias also carries b_fc * W_bg (the feedback matmul delivers the
    # raw fc output, without b_fc)
    b0d = (b0 + bfc * wbg).reshape(4, H).T.astype(np.float32)
    b1 = (b_ih_1 + b_hh_1).reshape(4, H).T.astype(np.float32)

    in_maps = []
    for i in range(N_CORES):
        sh = inputs[i * BSH : (i + 1) * BSH]  # [1024, 60, 8]
        x = np.ascontiguousarray(sh.transpose(2, 1, 0))  # [8, 60, 1024]
        # BG channel rides the feedback matmul for decode steps after the
        # first; the first decode step (t = T_ENC) keeps its real BG value
        x[0, T_ENC + 1 :, :] = 0.0
        in_maps.append(
            {
                "x": x.astype(F16),
                "w0x": w0x, "w0h": w0h, "w1x": w1x, "w1h": w1h,
                "wbgfc": wbgfc, "wfc": wfc,
                "b0e": b0e, "b0d": b0d, "b1": b1,
            }
        )

    res = bass_utils.run_bass_kernel_spmd(
        nc, in_maps, core_ids=list(range(N_CORES))
    )
    outs = []
    for i in range(N_CORES):
        o = res.results[i]["out"]  # [12, 1024] fp32 raw fc output
        outs.append(o.T[:, :, None])  # [1024, 12, 1]
    return (np.concatenate(outs, axis=0) + bfc).astype(np.float32)


if __name__ == "__main__":
    _get_nc()
    print("build + compile OK")
